# revision 11
# baseline (speedup 1.0000x reference)
"""Trainium2 Bass kernel for nn_CustomLoss_74826920231413.

Loss structure (B=32, E=1024, K=20):
    c  = complex(nnOutput[:, :NOUT], nnOutput[:, NOUT:])
    d  = c[:, :K];  U = c[:, K:VLOC].reshape(B,E,K);  V = c[:, VLOC:].reshape(B,E,K)
    obj1/obj2 = sum_{j<k} |U^T U| / B (no conj), same for V
    pred = U @ diag(d) @ V^T;  tk = complex(kern_real, kern_imag)
    loss = ||tk - pred||^2 / ||tk||^2 + 0.01*(obj1+obj2)

Device strategy (data-parallel over B, 4 batch rows per core, 8 cores):
    ||tk - pred||^2 = ||tk||^2 - 2*Re<conj(tk),pred> + ||pred||^2, so the
    device only needs one streaming pass over tk producing small outputs:
      * gram[b]  = [Ur|Ui]^T[Ur|Ui] and [Vr|Vi]^T[Vr|Vi]  -> objs, ||pred||^2
      * y[b]     = W^T tkr / W^T tki with W = [Ur|Ui]      -> cross term
      * den      = sum tk^2 partials
    Host assembles the three scalars from these partials in float64.

    All device inputs ride in fp8 e4m3 (validated: end-to-end loss error
    ~5e-4 vs the 2e-2 gate), halving the dominant HBM stream vs fp16.
    Inputs are split across BOTH HWDGE rings (sync: tkr, scalar: xuv+tki)
    and everything stays resident in SBUF (64KB/partition of 208).

    den = sum tk^2 is engine-bound at 8 bits (no DVE packing), so it is
    split three ways per (b, tensor): DVE stt-accum chunks, ACT
    Square-accum chunks, and a PE DoubleRow self-matmul whose [128,128]
    PSUM accumulates q^T q for diagonal f-blocks across ALL (b,t); its
    diagonal carries the remaining den partials.  The y matmuls use fp8
    DoubleRow (2 e-chunks per pass); even/odd b share one PSUM tile at
    partition offsets 0/64 so one fp16 evacuation serves two batch rows.
"""

import sys

for _p in ("/opt/trn_rl_repo", "/root/.axon_site/_ro/trn_rl_repo"):
    if _p not in sys.path:
        sys.path.append(_p)

import ml_dtypes
import numpy as np

import concourse.bacc as bacc
import concourse.mybir as mybir
import concourse.tile as tile
from concourse.bass_utils import run_bass_kernel_spmd

# Problem constants (hardcoded per harness contract)
E = 1024
K = 20
NOUT = K * (2 * E + 1)          # 40980
VLOC = K + K * E                # 20500
PENALTY = 0.01
B = 32
NCORES = 8
NB = B // NCORES                # batch rows per core
NPAIR = NB // 2                 # PSUM-sharing batch pairs
NCH = E // 128                  # 8 e-chunks of 128 partitions
F32 = mybir.dt.float32
F16 = mybir.dt.float16
F8 = mybir.dt.float8e4
NP_F8 = ml_dtypes.float8_e4m3   # TRN FP8_EXP4-compatible (max 240)

# per-(b,t) den chunk split, indexed by 2*b+t: (dve, act) leading chunks,
# PE takes the rest (must be even for DoubleRow pairs).  Tuned from
# measured rates: DVE 1.07us/chunk, ACT 0.93, PE ~0.68 marginal.
DEN_SPLIT = [
    (3, 3), (2, 4),   # b0 r, i
    (2, 4), (2, 4),   # b1
    (2, 4), (2, 4),   # b2
    (2, 4), (2, 2),   # b3
]

_PROGRAM_CACHE = {}


def _build_program():
    """Per-core SPMD Bass program. Same program on all 8 cores; each core
    receives its own 4-row slice of the inputs (host-packed layouts)."""
    nc = bacc.Bacc("TRN2", target_bir_lowering=False, debug=False)

    # host-packed [Ur|Ui|Vr|Vi] fp8, partition-major outer: [p, b, c, 80]
    xuv_d = nc.dram_tensor("xuv", [128, NB, NCH, 80], F8, kind="ExternalInput").ap()
    # host-packed fp8 kernels, partition-major outer: [p, 2b+t, c, f],
    # e = c*128+p.  8KB contiguous per partition per (b,t) slice.
    qk_d = nc.dram_tensor("qk", [128, 2 * NB, NCH, E], F8, kind="ExternalInput").ap()

    gram_d = nc.dram_tensor("gram", [80, NB * 80], F32, kind="ExternalOutput").ap()
    ys_d = nc.dram_tensor("ys", [NB, 2, 40, E], F16, kind="ExternalOutput").ap()
    # merged den partials: cols 0:8 DVE, 8:16 ACT, 16:144 PE psum image
    deno_d = nc.dram_tensor("deno", [128, 144], F32, kind="ExternalOutput").ap()

    mult = mybir.AluOpType.mult
    Square = mybir.ActivationFunctionType.Square
    DR = mybir.MatmulPerfMode.DoubleRow

    n_pe_mm = sum((NCH - dv - da) // 2 for dv, da in DEN_SPLIT) * NCH
    with tile.TileContext(nc) as tc:
        with (
            tc.tile_pool(name="x", bufs=1) as xpool,
            tc.tile_pool(name="q", bufs=1) as qpool,
            tc.tile_pool(name="scr", bufs=2) as scrpool,
            tc.tile_pool(name="evac", bufs=2) as evacpool,
            tc.tile_pool(name="den", bufs=1) as denpool,
            tc.tile_pool(name="psg", bufs=1, space="PSUM") as psg_pool,
            tc.tile_pool(name="psyr", bufs=2, space="PSUM") as psyr_pool,
            tc.tile_pool(name="psyi", bufs=1, space="PSUM") as psyi_pool,
            tc.tile_pool(name="psd", bufs=1, space="PSUM") as psd_pool,
        ):
            # ---- input DMAs, all on the sync HWDGE ring in consumption
            # order; compute engines never dispatch input DMAs.
            x_sb = xpool.tile([128, NB, NCH, 80], F8, name="x")
            nc.sync.dma_start(x_sb[:], xuv_d)
            q_all = qpool.tile([128, 2 * NB, NCH, E], F8, name="q")
            for j in range(2 * NB):
                nc.sync.dma_start(q_all[:, j], qk_d[:, j])

            # ---- accumulators + ACT Square-table preload on a dummy
            den_o = denpool.tile([128, 144], F32, name="den_o")
            zz = denpool.tile([128, 1], F32, name="zz")
            nc.vector.memset(den_o[:, 0:16], 0.0)
            nc.vector.memset(zz[:], 0.0)
            zz2 = denpool.tile([128, 1], F32, name="zz2")
            nc.scalar.activation(zz2[:], zz[:], Square)
            ps_den = psd_pool.tile([128, 128], F32, name="ps_den")

            # ---- PE: all gram matmuls first (need only xuv).  One DR
            # matmul per (b, chunk-pair) over the full 80-col [U|V] block:
            # out[0:80, 0:80] holds S_U at [0:40,0:40], S_V at [40:80,40:80].
            pg = psg_pool.tile([80, NB * 80], F32, name="ps_g")
            for b in range(NB):
                gs = slice(80 * b, 80 * b + 80)
                for cp in range(NCH // 2):
                    xw = x_sb[:, b, 2 * cp:2 * cp + 2, :]
                    nc.tensor.matmul(
                        pg[:, gs], xw, xw,
                        start=cp == 0, stop=cp == NCH // 2 - 1, perf_mode=DR,
                    )
            g_ev = evacpool.tile([80, NB * 80], F32, name="g_ev")
            nc.vector.tensor_copy(g_ev[:], pg[:])
            nc.gpsimd.dma_start(gram_d, g_ev[:])

            pe_idx = 0
            for b in range(NB):
                pyr = psyr_pool.tile([64, E], F32, name="ps_yr")
                pyi = psyi_pool.tile([64, E], F32, name="ps_yi")
                for t in range(2):
                    j = 2 * b + t
                    src = q_all[:, j]
                    dv, da = DEN_SPLIT[j]
                    # ---- PE: y matmuls (stationary = [Ur|Ui] padded to 64
                    # cols with Vr columns; host ignores rows 40:64)
                    py = pyr if t == 0 else pyi
                    for cp in range(NCH // 2):
                        w = x_sb[:, b, 2 * cp:2 * cp + 2, 0:64]
                        for h in range(2):
                            fs = slice(h * 512, (h + 1) * 512)
                            nc.tensor.matmul(
                                py[:, fs], w,
                                src[:, 2 * cp:2 * cp + 2, fs],
                                start=cp == 0, stop=cp == NCH // 2 - 1,
                                perf_mode=DR,
                            )
                    # ---- DVE / ACT den chunks
                    scr_v = scrpool.tile([128, dv * E], F8, name="scr_v")
                    nc.vector.scalar_tensor_tensor(
                        scr_v[:], src[:, 0:dv, :], 1.0, src[:, 0:dv, :],
                        mult, mult, accum_out=den_o[:, j:j + 1],
                    )
                    scr_a = scrpool.tile([128, da * E], F8, name="scr_a")
                    nc.scalar.activation(
                        scr_a[:], src[:, dv:dv + da, :], Square,
                        accum_out=den_o[:, 8 + j:9 + j],
                    )
                    # ---- PE den: DoubleRow self-matmuls accumulating into
                    # one [128,128] PSUM whose diagonal carries the partials
                    for c0 in range(dv + da, NCH, 2):
                        for fb in range(NCH):
                            fs = slice(fb * 128, (fb + 1) * 128)
                            qq = src[:, c0:c0 + 2, fs]
                            nc.tensor.matmul(
                                ps_den[:, :], qq, qq,
                                start=pe_idx == 0, stop=pe_idx == n_pe_mm - 1,
                                perf_mode=DR, skip_group_check=True,
                            )
                            pe_idx += 1
                    # ---- evacuation on DVE (ACT runs den only), fp16
                    yv = evacpool.tile([40, E], F16, name=f"y_ev{t}")
                    nc.vector.tensor_copy(yv[:], py[0:40, :])
                    nc.gpsimd.dma_start(ys_d[b, t], yv[:])

            nc.vector.tensor_copy(den_o[:, 16:144], ps_den[:])
            nc.sync.dma_start(deno_d, den_o[:])

    nc.compile()
    return nc


def _get_program():
    if "nc" not in _PROGRAM_CACHE:
        _PROGRAM_CACHE["nc"] = _build_program()
    return _PROGRAM_CACHE["nc"]


def _to_fp8(x):
    return np.clip(x, -240.0, 240.0).astype(NP_F8)


def _pack_inputs(nn, tkr, tki):
    """Host-side packing: per-core input dicts with device-friendly layouts."""
    # partition-major outer fp8: [B, E, E] -> [B, p, c, f] with e = c*128+p,
    # then per core: [p, 2b+t, c, f]
    q8r = _to_fp8(tkr).reshape(B, NCH, 128, E)
    q8i = _to_fp8(tki).reshape(B, NCH, 128, E)
    qk = np.empty((NCORES, 128, 2 * NB, NCH, E), dtype=NP_F8)
    for b in range(NB):
        for i in range(NCORES):
            qk[i, :, 2 * b] = q8r[i * NB + b].transpose(1, 0, 2)
            qk[i, :, 2 * b + 1] = q8i[i * NB + b].transpose(1, 0, 2)
    # [B, E, K] slices of nn
    Ur = nn[:, K:VLOC].reshape(B, E, K)
    Ui = nn[:, NOUT + K:NOUT + VLOC].reshape(B, E, K)
    Vr = nn[:, VLOC:NOUT].reshape(B, E, K)
    Vi = nn[:, NOUT + VLOC:2 * NOUT].reshape(B, E, K)
    xuv = np.concatenate([Ur, Ui, Vr, Vi], axis=2)        # [B, E, 80] f32
    # [B, p, c, 80] -> per core [p, b, c, 80]
    xuv = _to_fp8(xuv.reshape(B, NCH, 128, 80).transpose(0, 2, 1, 3))
    xuv = xuv.reshape(NCORES, NB, 128, NCH, 80).transpose(0, 2, 1, 3, 4)
    return [
        {"xuv": np.ascontiguousarray(xuv[i]), "qk": qk[i]}
        for i in range(NCORES)
    ]


def _run_device(nn, tkr, tki, trace=False):
    nc = _get_program()
    in_maps = _pack_inputs(nn, tkr, tki)
    return run_bass_kernel_spmd(nc, in_maps, list(range(NCORES)), trace=trace)


def _finalize(nn, results, batch_size):
    """Assemble (loss, obj1, obj2) from per-core device partials (float64)."""
    nn = np.asarray(nn)
    d = (nn[:, :K] + 1j * nn[:, NOUT:NOUT + K]).astype(np.complex128)
    Vr = nn[:, VLOC:NOUT].reshape(B, E, K).astype(np.float64)
    Vi = nn[:, NOUT + VLOC:2 * NOUT].reshape(B, E, K).astype(np.float64)
    V = Vr + 1j * Vi

    # unstack the pair-packed [NPAIR, 104, ...] outputs into per-b arrays
    # device gram block per b: [80,80] at cols 80b; S_U = [0:40,0:40],
    # S_V = [40:80, 40:80] of each block
    SU = np.empty((B, 40, 40), dtype=np.float64)
    SV = np.empty((B, 40, 40), dtype=np.float64)
    yr = np.empty((B, 40, E), dtype=np.float64)
    yi = np.empty((B, 40, E), dtype=np.float64)
    den = 0.0
    for i, r in enumerate(results):
        for b in range(NB):
            gb = i * NB + b
            g = r["gram"][:, 80 * b:80 * b + 80].astype(np.float64)
            SU[gb] = g[0:40, 0:40]
            SV[gb] = g[40:80, 40:80]
            yr[gb] = r["ys"][b, 0].astype(np.float64)
            yi[gb] = r["ys"][b, 1].astype(np.float64)
        den += float(np.sum(r["deno"][:, 0:16], dtype=np.float64))
        den += float(np.trace(r["deno"][:, 16:144].astype(np.float64)))

    Srr = SU[:, 0:20, 0:20]
    Sri = SU[:, 0:20, 20:40]
    Sii = SU[:, 20:40, 20:40]
    Trr = SV[:, 0:20, 0:20]
    Tri = SV[:, 0:20, 20:40]
    Tii = SV[:, 20:40, 20:40]
    SriT = np.transpose(Sri, (0, 2, 1))
    TriT = np.transpose(Tri, (0, 2, 1))
    G_U = (Srr - Sii) + 1j * (Sri + SriT)
    G_V = (Trr - Tii) + 1j * (Tri + TriT)
    H_U = (Srr + Sii) + 1j * (Sri - SriT)
    H_V = (Trr + Tii) + 1j * (Tri - TriT)

    mask = np.triu(np.ones((K, K), dtype=bool), k=1)
    bsz = float(batch_size)
    obj1 = float(np.sum(np.abs(G_U)[:, mask]) / bsz)
    obj2 = float(np.sum(np.abs(G_V)[:, mask]) / bsz)

    prednorm = float(
        np.real(
            np.einsum("bk,bl,bkl,bkl->", d, np.conj(d), np.conj(H_U), np.conj(H_V))
        )
    )

    # cross = Re<conj(tk), pred>; Wc[b,k,f] = sum_e conj(tk[e,f]) U[e,k]
    Wc = (yr[:, 0:20, :] + yi[:, 20:40, :]) + 1j * (yr[:, 20:40, :] - yi[:, 0:20, :])
    zeta = np.einsum("bfk,bkf->bk", V, Wc)
    cross = float(np.real(np.einsum("bk,bk->", d, zeta)))

    num = den - 2.0 * cross + prednorm
    loss = num / den + PENALTY * (obj1 + obj2)
    return (
        np.float32(loss),
        np.float32(obj1),
        np.float32(obj2),
    )


def kernel(nnOutput, kern_real, kern_imag, batch_Size):
    nn = np.ascontiguousarray(np.asarray(nnOutput, dtype=np.float32))
    tkr = np.asarray(kern_real, dtype=np.float32)
    tki = np.asarray(kern_imag, dtype=np.float32)
    res = _run_device(nn, tkr, tki).results
    return _finalize(nn, res, int(batch_Size))


# revision 12
# speedup vs baseline: 1.0862x; 1.0862x over previous
"""Trainium2 Bass kernel for nn_CustomLoss_74826920231413.

Loss structure (B=32, E=1024, K=20):
    c  = complex(nnOutput[:, :NOUT], nnOutput[:, NOUT:])
    d  = c[:, :K];  U = c[:, K:VLOC].reshape(B,E,K);  V = c[:, VLOC:].reshape(B,E,K)
    obj1/obj2 = sum_{j<k} |U^T U| / B (no conj), same for V
    pred = U @ diag(d) @ V^T;  tk = complex(kern_real, kern_imag)
    loss = ||tk - pred||^2 / ||tk||^2 + 0.01*(obj1+obj2)

Device strategy (data-parallel over B, 4 batch rows per core, 8 cores):
    ||tk - pred||^2 = ||tk||^2 - 2*Re<conj(tk),pred> + ||pred||^2, so the
    device only needs one streaming pass over tk producing small outputs:
      * gram[b]  = [Ur|Ui]^T[Ur|Ui] and [Vr|Vi]^T[Vr|Vi]  -> objs, ||pred||^2
      * y[b]     = W^T tkr / W^T tki with W = [Ur|Ui]      -> cross term
      * den      = sum tk^2 partials
    Host assembles the three scalars from these partials in float64.

    All device inputs ride in fp8 e4m3 (validated: end-to-end loss error
    ~5e-4 vs the 2e-2 gate), halving the dominant HBM stream vs fp16.
    Inputs are split across BOTH HWDGE rings (sync: tkr, scalar: xuv+tki)
    and everything stays resident in SBUF (64KB/partition of 208).

    den = sum tk^2 is engine-bound at 8 bits (no DVE packing), so it is
    split three ways per (b, tensor): DVE stt-accum chunks, ACT
    Square-accum chunks, and a PE DoubleRow self-matmul whose [128,128]
    PSUM accumulates q^T q for diagonal f-blocks across ALL (b,t); its
    diagonal carries the remaining den partials.  The y matmuls use fp8
    DoubleRow (2 e-chunks per pass); even/odd b share one PSUM tile at
    partition offsets 0/64 so one fp16 evacuation serves two batch rows.
"""

import sys

for _p in ("/opt/trn_rl_repo", "/root/.axon_site/_ro/trn_rl_repo"):
    if _p not in sys.path:
        sys.path.append(_p)

import ml_dtypes
import numpy as np

import concourse.bacc as bacc
import concourse.mybir as mybir
import concourse.tile as tile
from concourse.bass_utils import run_bass_kernel_spmd

# Problem constants (hardcoded per harness contract)
E = 1024
K = 20
NOUT = K * (2 * E + 1)          # 40980
VLOC = K + K * E                # 20500
PENALTY = 0.01
B = 32
NCORES = 8
NB = B // NCORES                # batch rows per core
NPAIR = NB // 2                 # PSUM-sharing batch pairs
NCH = E // 128                  # 8 e-chunks of 128 partitions
F32 = mybir.dt.float32
F16 = mybir.dt.float16
F8 = mybir.dt.float8e4
NP_F8 = ml_dtypes.float8_e4m3   # TRN FP8_EXP4-compatible (max 240)

# per-(b,t) den chunk split, indexed by 2*b+t: (dve, act) leading chunks,
# PE takes the rest (must be even for DoubleRow pairs).  Tuned from
# measured rates: DVE 1.07us/chunk, ACT 0.93, PE ~0.68 marginal.
DEN_SPLIT = [
    (3, 3), (3, 3),   # b0 r, i
    (3, 5), (3, 3),   # b1
    (3, 5), (3, 3),   # b2
    (3, 3), (2, 2),   # b3
]

_PROGRAM_CACHE = {}


def _build_program():
    """Per-core SPMD Bass program. Same program on all 8 cores; each core
    receives its own 4-row slice of the inputs (host-packed layouts)."""
    nc = bacc.Bacc("TRN2", target_bir_lowering=False, debug=False)

    # host-packed [Ur|Ui|Vr|Vi] fp8, partition-major outer: [p, b, c, 80]
    xuv_d = nc.dram_tensor("xuv", [128, NB, NCH, 80], F8, kind="ExternalInput").ap()
    # host-packed fp8 kernels, partition-major outer: [p, 2b+t, c, f],
    # e = c*128+p.  8KB contiguous per partition per (b,t) slice.
    qk_d = nc.dram_tensor("qk", [128, 2 * NB, NCH, E], F8, kind="ExternalInput").ap()

    gram_d = nc.dram_tensor("gram", [80, NB * 80], F32, kind="ExternalOutput").ap()
    ys_d = nc.dram_tensor("ys", [NB, 2, 40, E], F16, kind="ExternalOutput").ap()
    # merged den partials: cols 0:8 DVE, 8:16 ACT, 16:144 PE psum image
    deno_d = nc.dram_tensor("deno", [128, 144], F32, kind="ExternalOutput").ap()

    mult = mybir.AluOpType.mult
    Square = mybir.ActivationFunctionType.Square
    DR = mybir.MatmulPerfMode.DoubleRow

    n_pe_mm = sum((NCH - dv - da) // 2 for dv, da in DEN_SPLIT) * NCH
    with tile.TileContext(nc) as tc:
        with (
            tc.tile_pool(name="x", bufs=1) as xpool,
            tc.tile_pool(name="q", bufs=1) as qpool,
            tc.tile_pool(name="scr", bufs=2) as scrpool,
            tc.tile_pool(name="evac", bufs=2) as evacpool,
            tc.tile_pool(name="den", bufs=1) as denpool,
            tc.tile_pool(name="psg", bufs=1, space="PSUM") as psg_pool,
            tc.tile_pool(name="psyr", bufs=2, space="PSUM") as psyr_pool,
            tc.tile_pool(name="psyi", bufs=1, space="PSUM") as psyi_pool,
            tc.tile_pool(name="psd", bufs=1, space="PSUM") as psd_pool,
        ):
            # ---- input DMAs, all on the sync HWDGE ring in consumption
            # order; compute engines never dispatch input DMAs.
            x_sb = xpool.tile([128, NB, NCH, 80], F8, name="x")
            nc.sync.dma_start(x_sb[:], xuv_d)
            q_all = qpool.tile([128, 2 * NB, NCH, E], F8, name="q")
            for j in range(2 * NB):
                nc.sync.dma_start(q_all[:, j], qk_d[:, j])

            # ---- accumulators + ACT Square-table preload on a dummy
            den_o = denpool.tile([128, 144], F32, name="den_o")
            zz = denpool.tile([128, 1], F32, name="zz")
            nc.vector.memset(den_o[:, 0:16], 0.0)
            nc.vector.memset(zz[:], 0.0)
            zz2 = denpool.tile([128, 1], F32, name="zz2")
            nc.scalar.activation(zz2[:], zz[:], Square)
            ps_den = psd_pool.tile([128, 128], F32, name="ps_den")

            # ---- PE: all gram matmuls first (need only xuv).  One DR
            # matmul per (b, chunk-pair) over the full 80-col [U|V] block:
            # out[0:80, 0:80] holds S_U at [0:40,0:40], S_V at [40:80,40:80].
            pg = psg_pool.tile([80, NB * 80], F32, name="ps_g")
            for b in range(NB):
                gs = slice(80 * b, 80 * b + 80)
                for cp in range(NCH // 2):
                    xw = x_sb[:, b, 2 * cp:2 * cp + 2, :]
                    nc.tensor.matmul(
                        pg[:, gs], xw, xw,
                        start=cp == 0, stop=cp == NCH // 2 - 1, perf_mode=DR,
                    )
            g_ev = evacpool.tile([80, NB * 80], F32, name="g_ev")
            nc.vector.tensor_copy(g_ev[:], pg[:])
            nc.gpsimd.dma_start(gram_d, g_ev[:])

            pe_idx = 0
            for b in range(NB):
                pyr = psyr_pool.tile([64, E], F32, name="ps_yr")
                pyi = psyi_pool.tile([64, E], F32, name="ps_yi")
                for t in range(2):
                    j = 2 * b + t
                    src = q_all[:, j]
                    dv, da = DEN_SPLIT[j]
                    # ---- PE: y matmuls (stationary = [Ur|Ui] padded to 64
                    # cols with Vr columns; host ignores rows 40:64)
                    py = pyr if t == 0 else pyi
                    for cp in range(NCH // 2):
                        w = x_sb[:, b, 2 * cp:2 * cp + 2, 0:64]
                        for h in range(2):
                            fs = slice(h * 512, (h + 1) * 512)
                            nc.tensor.matmul(
                                py[:, fs], w,
                                src[:, 2 * cp:2 * cp + 2, fs],
                                start=cp == 0, stop=cp == NCH // 2 - 1,
                                perf_mode=DR,
                            )
                    # ---- DVE / ACT den chunks
                    scr_v = scrpool.tile([128, dv * E], F8, name="scr_v")
                    nc.vector.scalar_tensor_tensor(
                        scr_v[:], src[:, 0:dv, :], 1.0, src[:, 0:dv, :],
                        mult, mult, accum_out=den_o[:, j:j + 1],
                    )
                    scr_a = scrpool.tile([128, da * E], F8, name="scr_a")
                    nc.scalar.activation(
                        scr_a[:], src[:, dv:dv + da, :], Square,
                        accum_out=den_o[:, 8 + j:9 + j],
                    )
                    # ---- PE den: DoubleRow self-matmuls accumulating into
                    # one [128,128] PSUM whose diagonal carries the partials
                    for c0 in range(dv + da, NCH, 2):
                        for fb in range(NCH):
                            fs = slice(fb * 128, (fb + 1) * 128)
                            qq = src[:, c0:c0 + 2, fs]
                            nc.tensor.matmul(
                                ps_den[:, :], qq, qq,
                                start=pe_idx == 0, stop=pe_idx == n_pe_mm - 1,
                                perf_mode=DR, skip_group_check=True,
                            )
                            pe_idx += 1
                    # ---- evacuations (DVE: yr, ACT: yi), fp16, rows 0:40
                    if t == 0:
                        yv = evacpool.tile([40, E], F16, name="yr_ev")
                        nc.vector.tensor_copy(yv[:], pyr[0:40, :])
                        nc.gpsimd.dma_start(ys_d[b, 0], yv[:])
                    else:
                        ya = evacpool.tile([40, E], F16, name="yi_ev")
                        nc.scalar.copy(ya[:], pyi[0:40, :])
                        nc.gpsimd.dma_start(ys_d[b, 1], ya[:])

            nc.vector.tensor_copy(den_o[:, 16:144], ps_den[:])
            nc.sync.dma_start(deno_d, den_o[:])

    nc.compile()
    return nc


def _get_program():
    if "nc" not in _PROGRAM_CACHE:
        _PROGRAM_CACHE["nc"] = _build_program()
    return _PROGRAM_CACHE["nc"]


def _to_fp8(x):
    return np.clip(x, -240.0, 240.0).astype(NP_F8)


def _pack_inputs(nn, tkr, tki):
    """Host-side packing: per-core input dicts with device-friendly layouts."""
    # partition-major outer fp8: [B, E, E] -> [B, p, c, f] with e = c*128+p,
    # then per core: [p, 2b+t, c, f]
    q8r = _to_fp8(tkr).reshape(B, NCH, 128, E)
    q8i = _to_fp8(tki).reshape(B, NCH, 128, E)
    qk = np.empty((NCORES, 128, 2 * NB, NCH, E), dtype=NP_F8)
    for b in range(NB):
        for i in range(NCORES):
            qk[i, :, 2 * b] = q8r[i * NB + b].transpose(1, 0, 2)
            qk[i, :, 2 * b + 1] = q8i[i * NB + b].transpose(1, 0, 2)
    # [B, E, K] slices of nn
    Ur = nn[:, K:VLOC].reshape(B, E, K)
    Ui = nn[:, NOUT + K:NOUT + VLOC].reshape(B, E, K)
    Vr = nn[:, VLOC:NOUT].reshape(B, E, K)
    Vi = nn[:, NOUT + VLOC:2 * NOUT].reshape(B, E, K)
    xuv = np.concatenate([Ur, Ui, Vr, Vi], axis=2)        # [B, E, 80] f32
    # [B, p, c, 80] -> per core [p, b, c, 80]
    xuv = _to_fp8(xuv.reshape(B, NCH, 128, 80).transpose(0, 2, 1, 3))
    xuv = xuv.reshape(NCORES, NB, 128, NCH, 80).transpose(0, 2, 1, 3, 4)
    return [
        {"xuv": np.ascontiguousarray(xuv[i]), "qk": qk[i]}
        for i in range(NCORES)
    ]


def _run_device(nn, tkr, tki, trace=False):
    nc = _get_program()
    in_maps = _pack_inputs(nn, tkr, tki)
    return run_bass_kernel_spmd(nc, in_maps, list(range(NCORES)), trace=trace)


def _finalize(nn, results, batch_size):
    """Assemble (loss, obj1, obj2) from per-core device partials (float64)."""
    nn = np.asarray(nn)
    d = (nn[:, :K] + 1j * nn[:, NOUT:NOUT + K]).astype(np.complex128)
    Vr = nn[:, VLOC:NOUT].reshape(B, E, K).astype(np.float64)
    Vi = nn[:, NOUT + VLOC:2 * NOUT].reshape(B, E, K).astype(np.float64)
    V = Vr + 1j * Vi

    # unstack the pair-packed [NPAIR, 104, ...] outputs into per-b arrays
    # device gram block per b: [80,80] at cols 80b; S_U = [0:40,0:40],
    # S_V = [40:80, 40:80] of each block
    SU = np.empty((B, 40, 40), dtype=np.float64)
    SV = np.empty((B, 40, 40), dtype=np.float64)
    yr = np.empty((B, 40, E), dtype=np.float64)
    yi = np.empty((B, 40, E), dtype=np.float64)
    den = 0.0
    for i, r in enumerate(results):
        for b in range(NB):
            gb = i * NB + b
            g = r["gram"][:, 80 * b:80 * b + 80].astype(np.float64)
            SU[gb] = g[0:40, 0:40]
            SV[gb] = g[40:80, 40:80]
            yr[gb] = r["ys"][b, 0].astype(np.float64)
            yi[gb] = r["ys"][b, 1].astype(np.float64)
        den += float(np.sum(r["deno"][:, 0:16], dtype=np.float64))
        den += float(np.trace(r["deno"][:, 16:144].astype(np.float64)))

    Srr = SU[:, 0:20, 0:20]
    Sri = SU[:, 0:20, 20:40]
    Sii = SU[:, 20:40, 20:40]
    Trr = SV[:, 0:20, 0:20]
    Tri = SV[:, 0:20, 20:40]
    Tii = SV[:, 20:40, 20:40]
    SriT = np.transpose(Sri, (0, 2, 1))
    TriT = np.transpose(Tri, (0, 2, 1))
    G_U = (Srr - Sii) + 1j * (Sri + SriT)
    G_V = (Trr - Tii) + 1j * (Tri + TriT)
    H_U = (Srr + Sii) + 1j * (Sri - SriT)
    H_V = (Trr + Tii) + 1j * (Tri - TriT)

    mask = np.triu(np.ones((K, K), dtype=bool), k=1)
    bsz = float(batch_size)
    obj1 = float(np.sum(np.abs(G_U)[:, mask]) / bsz)
    obj2 = float(np.sum(np.abs(G_V)[:, mask]) / bsz)

    prednorm = float(
        np.real(
            np.einsum("bk,bl,bkl,bkl->", d, np.conj(d), np.conj(H_U), np.conj(H_V))
        )
    )

    # cross = Re<conj(tk), pred>; Wc[b,k,f] = sum_e conj(tk[e,f]) U[e,k]
    Wc = (yr[:, 0:20, :] + yi[:, 20:40, :]) + 1j * (yr[:, 20:40, :] - yi[:, 0:20, :])
    zeta = np.einsum("bfk,bkf->bk", V, Wc)
    cross = float(np.real(np.einsum("bk,bk->", d, zeta)))

    num = den - 2.0 * cross + prednorm
    loss = num / den + PENALTY * (obj1 + obj2)
    return (
        np.float32(loss),
        np.float32(obj1),
        np.float32(obj2),
    )


def kernel(nnOutput, kern_real, kern_imag, batch_Size):
    nn = np.ascontiguousarray(np.asarray(nnOutput, dtype=np.float32))
    tkr = np.asarray(kern_real, dtype=np.float32)
    tki = np.asarray(kern_imag, dtype=np.float32)
    res = _run_device(nn, tkr, tki).results
    return _finalize(nn, res, int(batch_Size))


# revision 13
# speedup vs baseline: 1.0906x; 1.0041x over previous
"""Trainium2 Bass kernel for nn_CustomLoss_74826920231413.

Loss structure (B=32, E=1024, K=20):
    c  = complex(nnOutput[:, :NOUT], nnOutput[:, NOUT:])
    d  = c[:, :K];  U = c[:, K:VLOC].reshape(B,E,K);  V = c[:, VLOC:].reshape(B,E,K)
    obj1/obj2 = sum_{j<k} |U^T U| / B (no conj), same for V
    pred = U @ diag(d) @ V^T;  tk = complex(kern_real, kern_imag)
    loss = ||tk - pred||^2 / ||tk||^2 + 0.01*(obj1+obj2)

Device strategy (data-parallel over B, 4 batch rows per core, 8 cores):
    ||tk - pred||^2 = ||tk||^2 - 2*Re<conj(tk),pred> + ||pred||^2, so the
    device only needs one streaming pass over tk producing small outputs:
      * gram[b]  = [Ur|Ui]^T[Ur|Ui] and [Vr|Vi]^T[Vr|Vi]  -> objs, ||pred||^2
      * y[b]     = W^T tkr / W^T tki with W = [Ur|Ui]      -> cross term
      * den      = sum tk^2 partials
    Host assembles the three scalars from these partials in float64.

    All device inputs ride in fp8 e4m3 (validated: end-to-end loss error
    ~5e-4 vs the 2e-2 gate), halving the dominant HBM stream vs fp16.
    Inputs are split across BOTH HWDGE rings (sync: tkr, scalar: xuv+tki)
    and everything stays resident in SBUF (64KB/partition of 208).

    den = sum tk^2 is engine-bound at 8 bits (no DVE packing), so it is
    split three ways per (b, tensor): DVE stt-accum chunks, ACT
    Square-accum chunks, and a PE DoubleRow self-matmul whose [128,128]
    PSUM accumulates q^T q for diagonal f-blocks across ALL (b,t); its
    diagonal carries the remaining den partials.  The y matmuls use fp8
    DoubleRow (2 e-chunks per pass); even/odd b share one PSUM tile at
    partition offsets 0/64 so one fp16 evacuation serves two batch rows.
"""

import sys

for _p in ("/opt/trn_rl_repo", "/root/.axon_site/_ro/trn_rl_repo"):
    if _p not in sys.path:
        sys.path.append(_p)

import ml_dtypes
import numpy as np

import concourse.bacc as bacc
import concourse.mybir as mybir
import concourse.tile as tile
from concourse.bass_utils import run_bass_kernel_spmd

# Problem constants (hardcoded per harness contract)
E = 1024
K = 20
NOUT = K * (2 * E + 1)          # 40980
VLOC = K + K * E                # 20500
PENALTY = 0.01
B = 32
NCORES = 8
NB = B // NCORES                # batch rows per core
NPAIR = NB // 2                 # PSUM-sharing batch pairs
NCH = E // 128                  # 8 e-chunks of 128 partitions
F32 = mybir.dt.float32
F16 = mybir.dt.float16
F8 = mybir.dt.float8e4
NP_F8 = ml_dtypes.float8_e4m3   # TRN FP8_EXP4-compatible (max 240)

# per-(b,t) den chunk split, indexed by 2*b+t: (dve, act) leading chunks,
# PE takes the rest (must be even for DoubleRow pairs).  Tuned from
# measured rates: DVE 1.07us/chunk, ACT 0.93, PE ~0.68 marginal.
DEN_SPLIT = [
    (3, 3), (3, 3),   # b0 r, i
    (2, 4), (3, 3),   # b1
    (2, 4), (3, 3),   # b2
    (3, 3), (3, 3),   # b3
]

_PROGRAM_CACHE = {}


def _build_program():
    """Per-core SPMD Bass program. Same program on all 8 cores; each core
    receives its own 4-row slice of the inputs (host-packed layouts)."""
    nc = bacc.Bacc("TRN2", target_bir_lowering=False, debug=False)

    # host-packed [Ur|Ui|Vr|Vi] fp8, partition-major outer: [p, b, c, 80]
    xuv_d = nc.dram_tensor("xuv", [128, NB, NCH, 80], F8, kind="ExternalInput").ap()
    # host-packed fp8 kernels, partition-major outer: [p, 2b+t, c, f],
    # e = c*128+p.  8KB contiguous per partition per (b,t) slice.
    qk_d = nc.dram_tensor("qk", [128, 2 * NB, NCH, E], F8, kind="ExternalInput").ap()

    gram_d = nc.dram_tensor("gram", [80, NB * 80], F32, kind="ExternalOutput").ap()
    ys_d = nc.dram_tensor("ys", [NB, 2, 40, E], F16, kind="ExternalOutput").ap()
    # merged den partials: cols 0:8 DVE, 8:16 ACT, 16:144 PE psum image
    deno_d = nc.dram_tensor("deno", [128, 144], F32, kind="ExternalOutput").ap()

    mult = mybir.AluOpType.mult
    Square = mybir.ActivationFunctionType.Square
    DR = mybir.MatmulPerfMode.DoubleRow

    n_pe_mm = sum((NCH - dv - da) // 2 for dv, da in DEN_SPLIT) * NCH
    with tile.TileContext(nc) as tc:
        with (
            tc.tile_pool(name="x", bufs=1) as xpool,
            tc.tile_pool(name="q", bufs=1) as qpool,
            tc.tile_pool(name="scr", bufs=2) as scrpool,
            tc.tile_pool(name="evac", bufs=2) as evacpool,
            tc.tile_pool(name="den", bufs=1) as denpool,
            tc.tile_pool(name="psg", bufs=1, space="PSUM") as psg_pool,
            tc.tile_pool(name="psyr", bufs=2, space="PSUM") as psyr_pool,
            tc.tile_pool(name="psyi", bufs=1, space="PSUM") as psyi_pool,
            tc.tile_pool(name="psd", bufs=1, space="PSUM") as psd_pool,
        ):
            # ---- input DMAs, all on the sync HWDGE ring in consumption
            # order; compute engines never dispatch input DMAs.
            x_sb = xpool.tile([128, NB, NCH, 80], F8, name="x")
            nc.sync.dma_start(x_sb[:], xuv_d)
            q_all = qpool.tile([128, 2 * NB, NCH, E], F8, name="q")
            nc.sync.dma_start(q_all[:, 0, 0:4], qk_d[:, 0, 0:4])
            nc.sync.dma_start(q_all[:, 0, 4:NCH], qk_d[:, 0, 4:NCH])
            for j in range(1, 2 * NB):
                nc.sync.dma_start(q_all[:, j], qk_d[:, j])

            # ---- accumulators + ACT Square-table preload on a dummy
            den_o = denpool.tile([128, 144], F32, name="den_o")
            zz = denpool.tile([128, 1], F32, name="zz")
            nc.vector.memset(den_o[:, 0:16], 0.0)
            nc.vector.memset(zz[:], 0.0)
            zz2 = denpool.tile([128, 1], F32, name="zz2")
            nc.scalar.activation(zz2[:], zz[:], Square)
            ps_den = psd_pool.tile([128, 128], F32, name="ps_den")

            # ---- PE: all gram matmuls first (need only xuv).  One DR
            # matmul per (b, chunk-pair) over the full 80-col [U|V] block:
            # out[0:80, 0:80] holds S_U at [0:40,0:40], S_V at [40:80,40:80].
            pg = psg_pool.tile([80, NB * 80], F32, name="ps_g")
            for b in range(NB):
                gs = slice(80 * b, 80 * b + 80)
                for cp in range(NCH // 2):
                    xw = x_sb[:, b, 2 * cp:2 * cp + 2, :]
                    nc.tensor.matmul(
                        pg[:, gs], xw, xw,
                        start=cp == 0, stop=cp == NCH // 2 - 1, perf_mode=DR,
                    )
            g_ev = evacpool.tile([80, NB * 80], F32, name="g_ev")
            nc.vector.tensor_copy(g_ev[:], pg[:])
            nc.gpsimd.dma_start(gram_d, g_ev[:])

            pe_idx = 0
            for b in range(NB):
                pyr = psyr_pool.tile([64, E], F32, name="ps_yr")
                pyi = psyi_pool.tile([64, E], F32, name="ps_yi")
                for t in range(2):
                    j = 2 * b + t
                    src = q_all[:, j]
                    dv, da = DEN_SPLIT[j]
                    # ---- PE: y matmuls (stationary = [Ur|Ui] padded to 64
                    # cols with Vr columns; host ignores rows 40:64)
                    py = pyr if t == 0 else pyi
                    for cp in range(NCH // 2):
                        w = x_sb[:, b, 2 * cp:2 * cp + 2, 0:64]
                        for h in range(2):
                            fs = slice(h * 512, (h + 1) * 512)
                            nc.tensor.matmul(
                                py[:, fs], w,
                                src[:, 2 * cp:2 * cp + 2, fs],
                                start=cp == 0, stop=cp == NCH // 2 - 1,
                                perf_mode=DR,
                            )
                    # ---- DVE / ACT den chunks
                    scr_v = scrpool.tile([128, dv * E], F8, name="scr_v")
                    nc.vector.scalar_tensor_tensor(
                        scr_v[:], src[:, 0:dv, :], 1.0, src[:, 0:dv, :],
                        mult, mult, accum_out=den_o[:, j:j + 1],
                    )
                    scr_a = scrpool.tile([128, da * E], F8, name="scr_a")
                    nc.scalar.activation(
                        scr_a[:], src[:, dv:dv + da, :], Square,
                        accum_out=den_o[:, 8 + j:9 + j],
                    )
                    # ---- PE den: DoubleRow self-matmuls accumulating into
                    # one [128,128] PSUM whose diagonal carries the partials
                    for c0 in range(dv + da, NCH, 2):
                        for fb in range(NCH):
                            fs = slice(fb * 128, (fb + 1) * 128)
                            qq = src[:, c0:c0 + 2, fs]
                            nc.tensor.matmul(
                                ps_den[:, :], qq, qq,
                                start=pe_idx == 0, stop=pe_idx == n_pe_mm - 1,
                                perf_mode=DR, skip_group_check=True,
                            )
                            pe_idx += 1
                    # ---- evacuations (DVE: yr, ACT: yi), fp16, rows 0:40
                    if t == 0:
                        yv = evacpool.tile([40, E], F16, name="yr_ev")
                        nc.vector.tensor_copy(yv[:], pyr[0:40, :])
                        nc.gpsimd.dma_start(ys_d[b, 0], yv[:])
                    else:
                        ya = evacpool.tile([40, E], F16, name="yi_ev")
                        nc.scalar.copy(ya[:], pyi[0:40, :])
                        nc.gpsimd.dma_start(ys_d[b, 1], ya[:])

            nc.scalar.copy(den_o[:, 16:144], ps_den[:])
            nc.sync.dma_start(deno_d, den_o[:])

    nc.compile()
    return nc


def _get_program():
    if "nc" not in _PROGRAM_CACHE:
        _PROGRAM_CACHE["nc"] = _build_program()
    return _PROGRAM_CACHE["nc"]


def _to_fp8(x):
    return np.clip(x, -240.0, 240.0).astype(NP_F8)


def _pack_inputs(nn, tkr, tki):
    """Host-side packing: per-core input dicts with device-friendly layouts."""
    # partition-major outer fp8: [B, E, E] -> [B, p, c, f] with e = c*128+p,
    # then per core: [p, 2b+t, c, f]
    q8r = _to_fp8(tkr).reshape(B, NCH, 128, E)
    q8i = _to_fp8(tki).reshape(B, NCH, 128, E)
    qk = np.empty((NCORES, 128, 2 * NB, NCH, E), dtype=NP_F8)
    for b in range(NB):
        for i in range(NCORES):
            qk[i, :, 2 * b] = q8r[i * NB + b].transpose(1, 0, 2)
            qk[i, :, 2 * b + 1] = q8i[i * NB + b].transpose(1, 0, 2)
    # [B, E, K] slices of nn
    Ur = nn[:, K:VLOC].reshape(B, E, K)
    Ui = nn[:, NOUT + K:NOUT + VLOC].reshape(B, E, K)
    Vr = nn[:, VLOC:NOUT].reshape(B, E, K)
    Vi = nn[:, NOUT + VLOC:2 * NOUT].reshape(B, E, K)
    xuv = np.concatenate([Ur, Ui, Vr, Vi], axis=2)        # [B, E, 80] f32
    # [B, p, c, 80] -> per core [p, b, c, 80]
    xuv = _to_fp8(xuv.reshape(B, NCH, 128, 80).transpose(0, 2, 1, 3))
    xuv = xuv.reshape(NCORES, NB, 128, NCH, 80).transpose(0, 2, 1, 3, 4)
    return [
        {"xuv": np.ascontiguousarray(xuv[i]), "qk": qk[i]}
        for i in range(NCORES)
    ]


def _run_device(nn, tkr, tki, trace=False):
    nc = _get_program()
    in_maps = _pack_inputs(nn, tkr, tki)
    return run_bass_kernel_spmd(nc, in_maps, list(range(NCORES)), trace=trace)


def _finalize(nn, results, batch_size):
    """Assemble (loss, obj1, obj2) from per-core device partials (float64)."""
    nn = np.asarray(nn)
    d = (nn[:, :K] + 1j * nn[:, NOUT:NOUT + K]).astype(np.complex128)
    Vr = nn[:, VLOC:NOUT].reshape(B, E, K).astype(np.float64)
    Vi = nn[:, NOUT + VLOC:2 * NOUT].reshape(B, E, K).astype(np.float64)
    V = Vr + 1j * Vi

    # unstack the pair-packed [NPAIR, 104, ...] outputs into per-b arrays
    # device gram block per b: [80,80] at cols 80b; S_U = [0:40,0:40],
    # S_V = [40:80, 40:80] of each block
    SU = np.empty((B, 40, 40), dtype=np.float64)
    SV = np.empty((B, 40, 40), dtype=np.float64)
    yr = np.empty((B, 40, E), dtype=np.float64)
    yi = np.empty((B, 40, E), dtype=np.float64)
    den = 0.0
    for i, r in enumerate(results):
        for b in range(NB):
            gb = i * NB + b
            g = r["gram"][:, 80 * b:80 * b + 80].astype(np.float64)
            SU[gb] = g[0:40, 0:40]
            SV[gb] = g[40:80, 40:80]
            yr[gb] = r["ys"][b, 0].astype(np.float64)
            yi[gb] = r["ys"][b, 1].astype(np.float64)
        den += float(np.sum(r["deno"][:, 0:16], dtype=np.float64))
        den += float(np.trace(r["deno"][:, 16:144].astype(np.float64)))

    Srr = SU[:, 0:20, 0:20]
    Sri = SU[:, 0:20, 20:40]
    Sii = SU[:, 20:40, 20:40]
    Trr = SV[:, 0:20, 0:20]
    Tri = SV[:, 0:20, 20:40]
    Tii = SV[:, 20:40, 20:40]
    SriT = np.transpose(Sri, (0, 2, 1))
    TriT = np.transpose(Tri, (0, 2, 1))
    G_U = (Srr - Sii) + 1j * (Sri + SriT)
    G_V = (Trr - Tii) + 1j * (Tri + TriT)
    H_U = (Srr + Sii) + 1j * (Sri - SriT)
    H_V = (Trr + Tii) + 1j * (Tri - TriT)

    mask = np.triu(np.ones((K, K), dtype=bool), k=1)
    bsz = float(batch_size)
    obj1 = float(np.sum(np.abs(G_U)[:, mask]) / bsz)
    obj2 = float(np.sum(np.abs(G_V)[:, mask]) / bsz)

    prednorm = float(
        np.real(
            np.einsum("bk,bl,bkl,bkl->", d, np.conj(d), np.conj(H_U), np.conj(H_V))
        )
    )

    # cross = Re<conj(tk), pred>; Wc[b,k,f] = sum_e conj(tk[e,f]) U[e,k]
    Wc = (yr[:, 0:20, :] + yi[:, 20:40, :]) + 1j * (yr[:, 20:40, :] - yi[:, 0:20, :])
    zeta = np.einsum("bfk,bkf->bk", V, Wc)
    cross = float(np.real(np.einsum("bk,bk->", d, zeta)))

    num = den - 2.0 * cross + prednorm
    loss = num / den + PENALTY * (obj1 + obj2)
    return (
        np.float32(loss),
        np.float32(obj1),
        np.float32(obj2),
    )


def kernel(nnOutput, kern_real, kern_imag, batch_Size):
    nn = np.ascontiguousarray(np.asarray(nnOutput, dtype=np.float32))
    tkr = np.asarray(kern_real, dtype=np.float32)
    tki = np.asarray(kern_imag, dtype=np.float32)
    res = _run_device(nn, tkr, tki).results
    return _finalize(nn, res, int(batch_Size))


# revision 14
# speedup vs baseline: 1.0948x; 1.0038x over previous
"""Trainium2 Bass kernel for nn_CustomLoss_74826920231413.

Loss structure (B=32, E=1024, K=20):
    c  = complex(nnOutput[:, :NOUT], nnOutput[:, NOUT:])
    d  = c[:, :K];  U = c[:, K:VLOC].reshape(B,E,K);  V = c[:, VLOC:].reshape(B,E,K)
    obj1/obj2 = sum_{j<k} |U^T U| / B (no conj), same for V
    pred = U @ diag(d) @ V^T;  tk = complex(kern_real, kern_imag)
    loss = ||tk - pred||^2 / ||tk||^2 + 0.01*(obj1+obj2)

Device strategy (data-parallel over B, 4 batch rows per core, 8 cores):
    ||tk - pred||^2 = ||tk||^2 - 2*Re<conj(tk),pred> + ||pred||^2, so the
    device only needs one streaming pass over tk producing small outputs:
      * gram[b]  = [Ur|Ui]^T[Ur|Ui] and [Vr|Vi]^T[Vr|Vi]  -> objs, ||pred||^2
      * y[b]     = W^T tkr / W^T tki with W = [Ur|Ui]      -> cross term
      * den      = sum tk^2 partials
    Host assembles the three scalars from these partials in float64.

    All device inputs ride in fp8 e4m3 (validated: end-to-end loss error
    ~5e-4 vs the 2e-2 gate), halving the dominant HBM stream vs fp16.
    Inputs are split across BOTH HWDGE rings (sync: tkr, scalar: xuv+tki)
    and everything stays resident in SBUF (64KB/partition of 208).

    den = sum tk^2 is engine-bound at 8 bits (no DVE packing), so it is
    split three ways per (b, tensor): DVE stt-accum chunks, ACT
    Square-accum chunks, and a PE DoubleRow self-matmul whose [128,128]
    PSUM accumulates q^T q for diagonal f-blocks across ALL (b,t); its
    diagonal carries the remaining den partials.  The y matmuls use fp8
    DoubleRow (2 e-chunks per pass); even/odd b share one PSUM tile at
    partition offsets 0/64 so one fp16 evacuation serves two batch rows.
"""

import sys

for _p in ("/opt/trn_rl_repo", "/root/.axon_site/_ro/trn_rl_repo"):
    if _p not in sys.path:
        sys.path.append(_p)

import ml_dtypes
import numpy as np

import concourse.bacc as bacc
import concourse.mybir as mybir
import concourse.tile as tile
from concourse.bass_utils import run_bass_kernel_spmd

# Problem constants (hardcoded per harness contract)
E = 1024
K = 20
NOUT = K * (2 * E + 1)          # 40980
VLOC = K + K * E                # 20500
PENALTY = 0.01
B = 32
NCORES = 8
NB = B // NCORES                # batch rows per core
NPAIR = NB // 2                 # PSUM-sharing batch pairs
NCH = E // 128                  # 8 e-chunks of 128 partitions
F32 = mybir.dt.float32
F16 = mybir.dt.float16
F8 = mybir.dt.float8e4
NP_F8 = ml_dtypes.float8_e4m3   # TRN FP8_EXP4-compatible (max 240)

# per-(b,t) den chunk split, indexed by 2*b+t: (dve, act) leading chunks,
# PE takes the rest (must be even for DoubleRow pairs).  Tuned from
# measured rates: DVE 1.07us/chunk, ACT 0.93, PE ~0.68 marginal.
DEN_SPLIT = [
    (3, 3), (3, 3),   # b0 r, i
    (2, 4), (3, 3),   # b1
    (2, 4), (3, 3),   # b2
    (3, 3), (3, 3),   # b3
]

_PROGRAM_CACHE = {}


def _build_program():
    """Per-core SPMD Bass program. Same program on all 8 cores; each core
    receives its own 4-row slice of the inputs (host-packed layouts)."""
    nc = bacc.Bacc("TRN2", target_bir_lowering=False, debug=False)

    # host-packed [Ur|Ui|Vr|Vi] fp8, partition-major outer: [p, b, c, 80]
    xuv_d = nc.dram_tensor("xuv", [128, NB, NCH, 80], F8, kind="ExternalInput").ap()
    # host-packed fp8 kernels, partition-major outer: [p, 2b+t, c, f],
    # e = c*128+p.  8KB contiguous per partition per (b,t) slice.
    qk_d = nc.dram_tensor("qk", [128, 2 * NB, NCH, E], F8, kind="ExternalInput").ap()

    gram_d = nc.dram_tensor("gram", [80, NB * 80], F32, kind="ExternalOutput").ap()
    ys_d = nc.dram_tensor("ys", [NB, 2, 40, E], F16, kind="ExternalOutput").ap()
    denv_d = nc.dram_tensor("denv", [128, 2 * NB], F32, kind="ExternalOutput").ap()
    dena_d = nc.dram_tensor("dena", [128, 2 * NB], F32, kind="ExternalOutput").ap()
    denp_d = nc.dram_tensor("denp", [128, 128], F32, kind="ExternalOutput").ap()

    mult = mybir.AluOpType.mult
    Square = mybir.ActivationFunctionType.Square
    DR = mybir.MatmulPerfMode.DoubleRow

    n_pe_mm = sum((NCH - dv - da) // 2 for dv, da in DEN_SPLIT) * NCH
    with tile.TileContext(nc) as tc:
        with (
            tc.tile_pool(name="x", bufs=1) as xpool,
            tc.tile_pool(name="q", bufs=1) as qpool,
            tc.tile_pool(name="scr", bufs=2) as scrpool,
            tc.tile_pool(name="evac", bufs=2) as evacpool,
            tc.tile_pool(name="den", bufs=1) as denpool,
            tc.tile_pool(name="psg", bufs=1, space="PSUM") as psg_pool,
            tc.tile_pool(name="psyr", bufs=2, space="PSUM") as psyr_pool,
            tc.tile_pool(name="psyi", bufs=1, space="PSUM") as psyi_pool,
            tc.tile_pool(name="psd", bufs=1, space="PSUM") as psd_pool,
        ):
            # ---- input DMAs, all on the sync HWDGE ring in consumption
            # order; compute engines never dispatch input DMAs.
            x_sb = xpool.tile([128, NB, NCH, 80], F8, name="x")
            nc.sync.dma_start(x_sb[:], xuv_d)
            q_all = qpool.tile([128, 2 * NB, NCH, E], F8, name="q")
            nc.sync.dma_start(q_all[:, 0, 0:4], qk_d[:, 0, 0:4])
            nc.sync.dma_start(q_all[:, 0, 4:NCH], qk_d[:, 0, 4:NCH])
            for j in range(1, 2 * NB):
                nc.sync.dma_start(q_all[:, j], qk_d[:, j])

            # ---- per-engine accumulators (separate tiles: a shared tile
            # makes Tile serialize the cross-engine accumulator drains)
            den_v = denpool.tile([128, 2 * NB], F32, name="den_v")
            den_a = denpool.tile([128, 2 * NB], F32, name="den_a")
            zz = denpool.tile([128, 1], F32, name="zz")
            nc.vector.memset(den_v[:], 0.0)
            nc.scalar.copy(den_a[:], den_v[:])
            nc.vector.memset(zz[:], 0.0)
            zz2 = denpool.tile([128, 1], F32, name="zz2")
            nc.scalar.activation(zz2[:], zz[:], Square)
            ps_den = psd_pool.tile([128, 128], F32, name="ps_den")

            # ---- PE: all gram matmuls first (need only xuv).  One DR
            # matmul per (b, chunk-pair) over the full 80-col [U|V] block:
            # out[0:80, 0:80] holds S_U at [0:40,0:40], S_V at [40:80,40:80].
            pg = psg_pool.tile([80, NB * 80], F32, name="ps_g")
            for b in range(NB):
                gs = slice(80 * b, 80 * b + 80)
                for cp in range(NCH // 2):
                    xw = x_sb[:, b, 2 * cp:2 * cp + 2, :]
                    nc.tensor.matmul(
                        pg[:, gs], xw, xw,
                        start=cp == 0, stop=cp == NCH // 2 - 1, perf_mode=DR,
                    )
            g_ev = evacpool.tile([80, NB * 80], F32, name="g_ev")
            nc.vector.tensor_copy(g_ev[:], pg[:])
            nc.gpsimd.dma_start(gram_d, g_ev[:])

            pe_idx = 0
            for b in range(NB):
                pyr = psyr_pool.tile([64, E], F32, name="ps_yr")
                pyi = psyi_pool.tile([64, E], F32, name="ps_yi")
                for t in range(2):
                    j = 2 * b + t
                    src = q_all[:, j]
                    dv, da = DEN_SPLIT[j]
                    # ---- PE: y matmuls (stationary = [Ur|Ui] padded to 64
                    # cols with Vr columns; host ignores rows 40:64)
                    py = pyr if t == 0 else pyi
                    for cp in range(NCH // 2):
                        w = x_sb[:, b, 2 * cp:2 * cp + 2, 0:64]
                        for h in range(2):
                            fs = slice(h * 512, (h + 1) * 512)
                            nc.tensor.matmul(
                                py[:, fs], w,
                                src[:, 2 * cp:2 * cp + 2, fs],
                                start=cp == 0, stop=cp == NCH // 2 - 1,
                                perf_mode=DR,
                            )
                    # ---- DVE / ACT den chunks
                    scr_v = scrpool.tile([128, dv * E], F8, name="scr_v")
                    nc.vector.scalar_tensor_tensor(
                        scr_v[:], src[:, 0:dv, :], 1.0, src[:, 0:dv, :],
                        mult, mult, accum_out=den_v[:, j:j + 1],
                    )
                    scr_a = scrpool.tile([128, da * E], F8, name="scr_a")
                    nc.scalar.activation(
                        scr_a[:], src[:, dv:dv + da, :], Square,
                        accum_out=den_a[:, j:j + 1],
                    )
                    # ---- PE den: DoubleRow self-matmuls accumulating into
                    # one [128,128] PSUM whose diagonal carries the partials
                    for c0 in range(dv + da, NCH, 2):
                        for fb in range(NCH):
                            fs = slice(fb * 128, (fb + 1) * 128)
                            qq = src[:, c0:c0 + 2, fs]
                            nc.tensor.matmul(
                                ps_den[:, :], qq, qq,
                                start=pe_idx == 0, stop=pe_idx == n_pe_mm - 1,
                                perf_mode=DR, skip_group_check=True,
                            )
                            pe_idx += 1
                    # ---- evacuations (DVE: yr, ACT: yi), fp16, rows 0:40
                    if t == 0:
                        yv = evacpool.tile([40, E], F16, name="yr_ev")
                        nc.vector.tensor_copy(yv[:], pyr[0:40, :])
                        nc.gpsimd.dma_start(ys_d[b, 0], yv[:])
                    else:
                        ya = evacpool.tile([40, E], F16, name="yi_ev")
                        nc.scalar.copy(ya[:], pyi[0:40, :])
                        nc.gpsimd.dma_start(ys_d[b, 1], ya[:])

            dp_ev = evacpool.tile([128, 128], F32, name="dp_ev")
            nc.scalar.copy(dp_ev[:], ps_den[:])
            nc.sync.dma_start(denp_d, dp_ev[:])
            nc.sync.dma_start(denv_d, den_v[:])
            nc.sync.dma_start(dena_d, den_a[:])

    nc.compile()
    return nc


def _get_program():
    if "nc" not in _PROGRAM_CACHE:
        _PROGRAM_CACHE["nc"] = _build_program()
    return _PROGRAM_CACHE["nc"]


def _to_fp8(x):
    return np.clip(x, -240.0, 240.0).astype(NP_F8)


def _pack_inputs(nn, tkr, tki):
    """Host-side packing: per-core input dicts with device-friendly layouts."""
    # partition-major outer fp8: [B, E, E] -> [B, p, c, f] with e = c*128+p,
    # then per core: [p, 2b+t, c, f]
    q8r = _to_fp8(tkr).reshape(B, NCH, 128, E)
    q8i = _to_fp8(tki).reshape(B, NCH, 128, E)
    qk = np.empty((NCORES, 128, 2 * NB, NCH, E), dtype=NP_F8)
    for b in range(NB):
        for i in range(NCORES):
            qk[i, :, 2 * b] = q8r[i * NB + b].transpose(1, 0, 2)
            qk[i, :, 2 * b + 1] = q8i[i * NB + b].transpose(1, 0, 2)
    # [B, E, K] slices of nn
    Ur = nn[:, K:VLOC].reshape(B, E, K)
    Ui = nn[:, NOUT + K:NOUT + VLOC].reshape(B, E, K)
    Vr = nn[:, VLOC:NOUT].reshape(B, E, K)
    Vi = nn[:, NOUT + VLOC:2 * NOUT].reshape(B, E, K)
    xuv = np.concatenate([Ur, Ui, Vr, Vi], axis=2)        # [B, E, 80] f32
    # [B, p, c, 80] -> per core [p, b, c, 80]
    xuv = _to_fp8(xuv.reshape(B, NCH, 128, 80).transpose(0, 2, 1, 3))
    xuv = xuv.reshape(NCORES, NB, 128, NCH, 80).transpose(0, 2, 1, 3, 4)
    return [
        {"xuv": np.ascontiguousarray(xuv[i]), "qk": qk[i]}
        for i in range(NCORES)
    ]


def _run_device(nn, tkr, tki, trace=False):
    nc = _get_program()
    in_maps = _pack_inputs(nn, tkr, tki)
    return run_bass_kernel_spmd(nc, in_maps, list(range(NCORES)), trace=trace)


def _finalize(nn, results, batch_size):
    """Assemble (loss, obj1, obj2) from per-core device partials (float64)."""
    nn = np.asarray(nn)
    d = (nn[:, :K] + 1j * nn[:, NOUT:NOUT + K]).astype(np.complex128)
    Vr = nn[:, VLOC:NOUT].reshape(B, E, K).astype(np.float64)
    Vi = nn[:, NOUT + VLOC:2 * NOUT].reshape(B, E, K).astype(np.float64)
    V = Vr + 1j * Vi

    # unstack the pair-packed [NPAIR, 104, ...] outputs into per-b arrays
    # device gram block per b: [80,80] at cols 80b; S_U = [0:40,0:40],
    # S_V = [40:80, 40:80] of each block
    SU = np.empty((B, 40, 40), dtype=np.float64)
    SV = np.empty((B, 40, 40), dtype=np.float64)
    yr = np.empty((B, 40, E), dtype=np.float64)
    yi = np.empty((B, 40, E), dtype=np.float64)
    den = 0.0
    for i, r in enumerate(results):
        for b in range(NB):
            gb = i * NB + b
            g = r["gram"][:, 80 * b:80 * b + 80].astype(np.float64)
            SU[gb] = g[0:40, 0:40]
            SV[gb] = g[40:80, 40:80]
            yr[gb] = r["ys"][b, 0].astype(np.float64)
            yi[gb] = r["ys"][b, 1].astype(np.float64)
        den += float(np.sum(r["denv"], dtype=np.float64))
        den += float(np.sum(r["dena"], dtype=np.float64))
        den += float(np.trace(r["denp"].astype(np.float64)))

    Srr = SU[:, 0:20, 0:20]
    Sri = SU[:, 0:20, 20:40]
    Sii = SU[:, 20:40, 20:40]
    Trr = SV[:, 0:20, 0:20]
    Tri = SV[:, 0:20, 20:40]
    Tii = SV[:, 20:40, 20:40]
    SriT = np.transpose(Sri, (0, 2, 1))
    TriT = np.transpose(Tri, (0, 2, 1))
    G_U = (Srr - Sii) + 1j * (Sri + SriT)
    G_V = (Trr - Tii) + 1j * (Tri + TriT)
    H_U = (Srr + Sii) + 1j * (Sri - SriT)
    H_V = (Trr + Tii) + 1j * (Tri - TriT)

    mask = np.triu(np.ones((K, K), dtype=bool), k=1)
    bsz = float(batch_size)
    obj1 = float(np.sum(np.abs(G_U)[:, mask]) / bsz)
    obj2 = float(np.sum(np.abs(G_V)[:, mask]) / bsz)

    prednorm = float(
        np.real(
            np.einsum("bk,bl,bkl,bkl->", d, np.conj(d), np.conj(H_U), np.conj(H_V))
        )
    )

    # cross = Re<conj(tk), pred>; Wc[b,k,f] = sum_e conj(tk[e,f]) U[e,k]
    Wc = (yr[:, 0:20, :] + yi[:, 20:40, :]) + 1j * (yr[:, 20:40, :] - yi[:, 0:20, :])
    zeta = np.einsum("bfk,bkf->bk", V, Wc)
    cross = float(np.real(np.einsum("bk,bk->", d, zeta)))

    num = den - 2.0 * cross + prednorm
    loss = num / den + PENALTY * (obj1 + obj2)
    return (
        np.float32(loss),
        np.float32(obj1),
        np.float32(obj2),
    )


def kernel(nnOutput, kern_real, kern_imag, batch_Size):
    nn = np.ascontiguousarray(np.asarray(nnOutput, dtype=np.float32))
    tkr = np.asarray(kern_real, dtype=np.float32)
    tki = np.asarray(kern_imag, dtype=np.float32)
    res = _run_device(nn, tkr, tki).results
    return _finalize(nn, res, int(batch_Size))


# revision 15
# speedup vs baseline: 1.1786x; 1.0766x over previous
"""Trainium2 Bass kernel for nn_CustomLoss_74826920231413.

Loss structure (B=32, E=1024, K=20):
    c  = complex(nnOutput[:, :NOUT], nnOutput[:, NOUT:])
    d  = c[:, :K];  U = c[:, K:VLOC].reshape(B,E,K);  V = c[:, VLOC:].reshape(B,E,K)
    obj1/obj2 = sum_{j<k} |U^T U| / B (no conj), same for V
    pred = U @ diag(d) @ V^T;  tk = complex(kern_real, kern_imag)
    loss = ||tk - pred||^2 / ||tk||^2 + 0.01*(obj1+obj2)

Device strategy (data-parallel over B, 4 batch rows per core, 8 cores):
    ||tk - pred||^2 = ||tk||^2 - 2*Re<conj(tk),pred> + ||pred||^2, so the
    device only needs one streaming pass over tk producing small outputs:
      * gram[b]  = [Ur|Ui]^T[Ur|Ui] and [Vr|Vi]^T[Vr|Vi]  -> objs, ||pred||^2
      * y[b]     = W^T tkr / W^T tki with W = [Ur|Ui]      -> cross term
      * den      = sum tk^2 partials
    Host assembles the three scalars from these partials in float64.

    All device inputs ride in fp8 e4m3 (validated: end-to-end loss error
    ~5e-4 vs the 2e-2 gate), halving the dominant HBM stream vs fp16.
    Inputs are split across BOTH HWDGE rings (sync: tkr, scalar: xuv+tki)
    and everything stays resident in SBUF (64KB/partition of 208).

    den = sum tk^2 is engine-bound at 8 bits (no DVE packing), so it is
    split three ways per (b, tensor): DVE stt-accum chunks, ACT
    Square-accum chunks, and a PE DoubleRow self-matmul whose [128,128]
    PSUM accumulates q^T q for diagonal f-blocks across ALL (b,t); its
    diagonal carries the remaining den partials.  The y matmuls use fp8
    DoubleRow (2 e-chunks per pass); even/odd b share one PSUM tile at
    partition offsets 0/64 so one fp16 evacuation serves two batch rows.
"""

import sys

for _p in ("/opt/trn_rl_repo", "/root/.axon_site/_ro/trn_rl_repo"):
    if _p not in sys.path:
        sys.path.append(_p)

import ml_dtypes
import numpy as np

import concourse.bacc as bacc
import concourse.mybir as mybir
import concourse.tile as tile
from concourse.bass_utils import run_bass_kernel_spmd

# Problem constants (hardcoded per harness contract)
E = 1024
K = 20
NOUT = K * (2 * E + 1)          # 40980
VLOC = K + K * E                # 20500
PENALTY = 0.01
B = 32
NCORES = 8
NB = B // NCORES                # batch rows per core
NPAIR = NB // 2                 # PSUM-sharing batch pairs
NCH = E // 128                  # 8 e-chunks of 128 partitions
F32 = mybir.dt.float32
F16 = mybir.dt.float16
F8 = mybir.dt.float8e4
NP_F8 = ml_dtypes.float8_e4m3   # TRN FP8_EXP4-compatible (max 240)

# per-(b,t) den chunk split, indexed by 2*b+t: (dve, act) leading chunks,
# PE takes the rest (must be even for DoubleRow pairs).  Tuned from
# measured rates: DVE 1.07us/chunk, ACT 0.93, PE ~0.68 marginal.
DEN_SPLIT = [
    (3, 3), (3, 3),   # b0 r, i
    (2, 4), (3, 3),   # b1
    (2, 4), (3, 3),   # b2
    (2, 2), (2, 2),   # b3
]

_PROGRAM_CACHE = {}


def _build_program():
    """Per-core SPMD Bass program. Same program on all 8 cores; each core
    receives its own 4-row slice of the inputs (host-packed layouts)."""
    nc = bacc.Bacc("TRN2", target_bir_lowering=False, debug=False)

    # host-packed [Ur|Ui|Vr|Vi] fp8, partition-major outer: [p, b, c, 80]
    xuv_d = nc.dram_tensor("xuv", [128, NB, NCH, 80], F8, kind="ExternalInput").ap()
    # host-packed fp8 kernels, partition-major outer: [p, 2b+t, c, f],
    # e = c*128+p.  8KB contiguous per partition per (b,t) slice.
    qk_d = nc.dram_tensor("qk", [128, 2 * NB, NCH, E], F8, kind="ExternalInput").ap()

    gram_d = nc.dram_tensor("gram", [80, NB * 80], F32, kind="ExternalOutput").ap()
    ys_d = nc.dram_tensor("ys", [NB, 2, 40, E], F16, kind="ExternalOutput").ap()
    denv_d = nc.dram_tensor("denv", [128, 2 * NB], F32, kind="ExternalOutput").ap()
    dena_d = nc.dram_tensor("dena", [128, 2 * NB], F32, kind="ExternalOutput").ap()
    denp_d = nc.dram_tensor("denp", [128, 128], F32, kind="ExternalOutput").ap()

    mult = mybir.AluOpType.mult
    Square = mybir.ActivationFunctionType.Square
    DR = mybir.MatmulPerfMode.DoubleRow

    n_pe_mm = sum((NCH - dv - da) // 2 for dv, da in DEN_SPLIT) * NCH
    with tile.TileContext(nc) as tc:
        with (
            tc.tile_pool(name="x", bufs=1) as xpool,
            tc.tile_pool(name="q", bufs=1) as qpool,
            tc.tile_pool(name="scr", bufs=2) as scrpool,
            tc.tile_pool(name="evac", bufs=2) as evacpool,
            tc.tile_pool(name="den", bufs=1) as denpool,
            tc.tile_pool(name="psg", bufs=1, space="PSUM") as psg_pool,
            tc.tile_pool(name="psyr", bufs=2, space="PSUM") as psyr_pool,
            tc.tile_pool(name="psyi", bufs=1, space="PSUM") as psyi_pool,
            tc.tile_pool(name="psd", bufs=1, space="PSUM") as psd_pool,
        ):
            # ---- input DMAs, all on the sync HWDGE ring in consumption
            # order; compute engines never dispatch input DMAs.
            x_sb = xpool.tile([128, NB, NCH, 80], F8, name="x")
            nc.sync.dma_start(x_sb[:], xuv_d)
            q_all = qpool.tile([128, 2 * NB, NCH, E], F8, name="q")
            for j in range(2 * NB):
                nc.sync.dma_start(q_all[:, j], qk_d[:, j])

            # ---- per-engine accumulators (separate tiles: a shared tile
            # makes Tile serialize the cross-engine accumulator drains)
            den_v = denpool.tile([128, 2 * NB], F32, name="den_v")
            den_a = denpool.tile([128, 2 * NB], F32, name="den_a")
            zz = denpool.tile([128, 1], F32, name="zz")
            nc.vector.memset(den_v[:], 0.0)
            nc.scalar.copy(den_a[:], den_v[:])
            nc.vector.memset(zz[:], 0.0)
            zz2 = denpool.tile([128, 1], F32, name="zz2")
            nc.scalar.activation(zz2[:], zz[:], Square)
            ps_den = psd_pool.tile([128, 128], F32, name="ps_den")

            # ---- PE: all gram matmuls first (need only xuv).  One DR
            # matmul per (b, chunk-pair) over the full 80-col [U|V] block:
            # out[0:80, 0:80] holds S_U at [0:40,0:40], S_V at [40:80,40:80].
            pg = psg_pool.tile([80, NB * 80], F32, name="ps_g")
            for b in range(NB):
                gs = slice(80 * b, 80 * b + 80)
                for cp in range(NCH // 2):
                    xw = x_sb[:, b, 2 * cp:2 * cp + 2, :]
                    nc.tensor.matmul(
                        pg[:, gs], xw, xw,
                        start=cp == 0, stop=cp == NCH // 2 - 1, perf_mode=DR,
                    )
            g_ev = evacpool.tile([80, NB * 80], F32, name="g_ev")
            nc.vector.tensor_copy(g_ev[:], pg[:])
            nc.gpsimd.dma_start(gram_d, g_ev[:])

            pe_idx = 0
            for b in range(NB):
                pyr = psyr_pool.tile([64, E], F32, name="ps_yr")
                pyi = psyi_pool.tile([64, E], F32, name="ps_yi")
                for t in range(2):
                    j = 2 * b + t
                    src = q_all[:, j]
                    dv, da = DEN_SPLIT[j]
                    # ---- PE: y matmuls (stationary = [Ur|Ui] padded to 64
                    # cols with Vr columns; host ignores rows 40:64)
                    py = pyr if t == 0 else pyi
                    for cp in range(NCH // 2):
                        w = x_sb[:, b, 2 * cp:2 * cp + 2, 0:64]
                        for h in range(2):
                            fs = slice(h * 512, (h + 1) * 512)
                            nc.tensor.matmul(
                                py[:, fs], w,
                                src[:, 2 * cp:2 * cp + 2, fs],
                                start=cp == 0, stop=cp == NCH // 2 - 1,
                                perf_mode=DR,
                            )
                    # ---- DVE / ACT den chunks
                    scr_v = scrpool.tile([128, dv * E], F8, name="scr_v")
                    nc.vector.scalar_tensor_tensor(
                        scr_v[:], src[:, 0:dv, :], 1.0, src[:, 0:dv, :],
                        mult, mult, accum_out=den_v[:, j:j + 1],
                    )
                    scr_a = scrpool.tile([128, da * E], F8, name="scr_a")
                    nc.scalar.activation(
                        scr_a[:], src[:, dv:dv + da, :], Square,
                        accum_out=den_a[:, j:j + 1],
                    )
                    # ---- PE den: DoubleRow self-matmuls accumulating into
                    # one [128,128] PSUM whose diagonal carries the partials
                    for c0 in range(dv + da, NCH, 2):
                        for fb in range(NCH):
                            fs = slice(fb * 128, (fb + 1) * 128)
                            qq = src[:, c0:c0 + 2, fs]
                            nc.tensor.matmul(
                                ps_den[:, :], qq, qq,
                                start=pe_idx == 0, stop=pe_idx == n_pe_mm - 1,
                                perf_mode=DR, skip_group_check=True,
                            )
                            pe_idx += 1
                    # ---- evacuations (DVE: yr, ACT: yi), fp16, rows 0:40
                    if t == 0:
                        yv = evacpool.tile([40, E], F16, name="yr_ev")
                        nc.vector.tensor_copy(yv[:], pyr[0:40, :])
                        nc.gpsimd.dma_start(ys_d[b, 0], yv[:])
                    else:
                        ya = evacpool.tile([40, E], F16, name="yi_ev")
                        nc.scalar.copy(ya[:], pyi[0:40, :])
                        nc.gpsimd.dma_start(ys_d[b, 1], ya[:])

            dp_ev = evacpool.tile([128, 128], F32, name="dp_ev")
            nc.scalar.copy(dp_ev[:], ps_den[:])
            nc.sync.dma_start(denp_d, dp_ev[:])
            nc.sync.dma_start(denv_d, den_v[:])
            nc.sync.dma_start(dena_d, den_a[:])

    nc.compile()
    return nc


def _get_program():
    if "nc" not in _PROGRAM_CACHE:
        _PROGRAM_CACHE["nc"] = _build_program()
    return _PROGRAM_CACHE["nc"]


def _to_fp8(x):
    return np.clip(x, -240.0, 240.0).astype(NP_F8)


def _pack_inputs(nn, tkr, tki):
    """Host-side packing: per-core input dicts with device-friendly layouts."""
    # partition-major outer fp8: [B, E, E] -> [B, p, c, f] with e = c*128+p,
    # then per core: [p, 2b+t, c, f]
    q8r = _to_fp8(tkr).reshape(B, NCH, 128, E)
    q8i = _to_fp8(tki).reshape(B, NCH, 128, E)
    qk = np.empty((NCORES, 128, 2 * NB, NCH, E), dtype=NP_F8)
    for b in range(NB):
        for i in range(NCORES):
            qk[i, :, 2 * b] = q8r[i * NB + b].transpose(1, 0, 2)
            qk[i, :, 2 * b + 1] = q8i[i * NB + b].transpose(1, 0, 2)
    # [B, E, K] slices of nn
    Ur = nn[:, K:VLOC].reshape(B, E, K)
    Ui = nn[:, NOUT + K:NOUT + VLOC].reshape(B, E, K)
    Vr = nn[:, VLOC:NOUT].reshape(B, E, K)
    Vi = nn[:, NOUT + VLOC:2 * NOUT].reshape(B, E, K)
    xuv = np.concatenate([Ur, Ui, Vr, Vi], axis=2)        # [B, E, 80] f32
    # [B, p, c, 80] -> per core [p, b, c, 80]
    xuv = _to_fp8(xuv.reshape(B, NCH, 128, 80).transpose(0, 2, 1, 3))
    xuv = xuv.reshape(NCORES, NB, 128, NCH, 80).transpose(0, 2, 1, 3, 4)
    return [
        {"xuv": np.ascontiguousarray(xuv[i]), "qk": qk[i]}
        for i in range(NCORES)
    ]


def _run_device(nn, tkr, tki, trace=False):
    nc = _get_program()
    in_maps = _pack_inputs(nn, tkr, tki)
    return run_bass_kernel_spmd(nc, in_maps, list(range(NCORES)), trace=trace)


def _finalize(nn, results, batch_size):
    """Assemble (loss, obj1, obj2) from per-core device partials (float64)."""
    nn = np.asarray(nn)
    d = (nn[:, :K] + 1j * nn[:, NOUT:NOUT + K]).astype(np.complex128)
    Vr = nn[:, VLOC:NOUT].reshape(B, E, K).astype(np.float64)
    Vi = nn[:, NOUT + VLOC:2 * NOUT].reshape(B, E, K).astype(np.float64)
    V = Vr + 1j * Vi

    # unstack the pair-packed [NPAIR, 104, ...] outputs into per-b arrays
    # device gram block per b: [80,80] at cols 80b; S_U = [0:40,0:40],
    # S_V = [40:80, 40:80] of each block
    SU = np.empty((B, 40, 40), dtype=np.float64)
    SV = np.empty((B, 40, 40), dtype=np.float64)
    yr = np.empty((B, 40, E), dtype=np.float64)
    yi = np.empty((B, 40, E), dtype=np.float64)
    den = 0.0
    for i, r in enumerate(results):
        for b in range(NB):
            gb = i * NB + b
            g = r["gram"][:, 80 * b:80 * b + 80].astype(np.float64)
            SU[gb] = g[0:40, 0:40]
            SV[gb] = g[40:80, 40:80]
            yr[gb] = r["ys"][b, 0].astype(np.float64)
            yi[gb] = r["ys"][b, 1].astype(np.float64)
        den += float(np.sum(r["denv"], dtype=np.float64))
        den += float(np.sum(r["dena"], dtype=np.float64))
        den += float(np.trace(r["denp"].astype(np.float64)))

    Srr = SU[:, 0:20, 0:20]
    Sri = SU[:, 0:20, 20:40]
    Sii = SU[:, 20:40, 20:40]
    Trr = SV[:, 0:20, 0:20]
    Tri = SV[:, 0:20, 20:40]
    Tii = SV[:, 20:40, 20:40]
    SriT = np.transpose(Sri, (0, 2, 1))
    TriT = np.transpose(Tri, (0, 2, 1))
    G_U = (Srr - Sii) + 1j * (Sri + SriT)
    G_V = (Trr - Tii) + 1j * (Tri + TriT)
    H_U = (Srr + Sii) + 1j * (Sri - SriT)
    H_V = (Trr + Tii) + 1j * (Tri - TriT)

    mask = np.triu(np.ones((K, K), dtype=bool), k=1)
    bsz = float(batch_size)
    obj1 = float(np.sum(np.abs(G_U)[:, mask]) / bsz)
    obj2 = float(np.sum(np.abs(G_V)[:, mask]) / bsz)

    prednorm = float(
        np.real(
            np.einsum("bk,bl,bkl,bkl->", d, np.conj(d), np.conj(H_U), np.conj(H_V))
        )
    )

    # cross = Re<conj(tk), pred>; Wc[b,k,f] = sum_e conj(tk[e,f]) U[e,k]
    Wc = (yr[:, 0:20, :] + yi[:, 20:40, :]) + 1j * (yr[:, 20:40, :] - yi[:, 0:20, :])
    zeta = np.einsum("bfk,bkf->bk", V, Wc)
    cross = float(np.real(np.einsum("bk,bk->", d, zeta)))

    num = den - 2.0 * cross + prednorm
    loss = num / den + PENALTY * (obj1 + obj2)
    return (
        np.float32(loss),
        np.float32(obj1),
        np.float32(obj2),
    )


def kernel(nnOutput, kern_real, kern_imag, batch_Size):
    nn = np.ascontiguousarray(np.asarray(nnOutput, dtype=np.float32))
    tkr = np.asarray(kern_real, dtype=np.float32)
    tki = np.asarray(kern_imag, dtype=np.float32)
    res = _run_device(nn, tkr, tki).results
    return _finalize(nn, res, int(batch_Size))
